# revision 2
# baseline (speedup 1.0000x reference)
"""Distributed Trainium2 kernel for single-head attention with QKV projections.

Problem: x:[8,2048,1024] f32, Wq/Wk/Wv:[1024,1024], bq/bk/bv:[1024]
  q = x@Wq+bq ; k = x@Wk+bk ; v = x@Wv+bv
  out = softmax(q k^T / sqrt(1024)) v          -> [8,2048,1024] f32

Sharding: data-parallel over batch — one batch element per NeuronCore
(8 cores), weights replicated. No collectives needed.

Per-core pipeline (all bf16 matmul inputs, f32 PSUM accumulation):
  1. load x shard, cast bf16, PE-transpose to xT [d,s]
  2. QT = Wq^T @ xT, KT = Wk^T @ xT  (layout [d_out, s]),  V = x @ Wv  ([t, d])
  3. flash-style attention per 128-query block:
       scores psum = QT^T KT ; attn = exp(scores/32) via ACT (+row-sum accum)
       attn^T via PE transpose ; out = (attn^T)^T V scaled by 1/rowsum
"""
import numpy as np

import concourse.bass as bass
import concourse.tile as tile
from concourse import bacc, mybir
from concourse.bass_utils import run_bass_kernel_spmd
from concourse.masks import make_identity

B, S, D = 8, 2048, 1024
P = 128
SO = S // P          # 16 token chunks of 128
DO = D // P          # 8 dim chunks of 128
NS = 512             # matmul moving free-dim / PSUM bank width (f32)
N_CORES = 8
SCALE = 1.0 / float(np.sqrt(np.float32(D)))

F32 = mybir.dt.float32
BF16 = mybir.dt.bfloat16


def build(with_bias: bool):
    nc = bacc.Bacc("TRN2", target_bir_lowering=False, debug=False,
                   num_devices=N_CORES)
    x_ext = nc.dram_tensor("x", [S, D], F32, kind="ExternalInput")
    w_ext = {
        "q": nc.dram_tensor("Wq", [D, D], F32, kind="ExternalInput"),
        "k": nc.dram_tensor("Wk", [D, D], F32, kind="ExternalInput"),
        "v": nc.dram_tensor("Wv", [D, D], F32, kind="ExternalInput"),
    }
    b_ext = {
        "q": nc.dram_tensor("bq", [1, D], F32, kind="ExternalInput"),
        "k": nc.dram_tensor("bk", [1, D], F32, kind="ExternalInput"),
        "v": nc.dram_tensor("bv", [1, D], F32, kind="ExternalInput"),
    }
    out_ext = nc.dram_tensor("out", [S, D], F32, kind="ExternalOutput")

    with tile.TileContext(nc) as tc:
        with (
            tc.tile_pool(name="persist", bufs=1) as persist,
            tc.tile_pool(name="psum_mm", bufs=4, space="PSUM") as psum_mm,
            tc.tile_pool(name="psum_tr", bufs=2, space="PSUM") as psum_tr,
        ):
            ident = persist.tile([P, P], BF16, tag="ident")
            make_identity(nc, ident)

            # QKV outputs (persist across both phases)
            QT = persist.tile([P, DO, S], BF16, tag="QT")   # [d_out, s]
            KT = persist.tile([P, DO, S], BF16, tag="KT")   # [d_out, s]
            V = persist.tile([P, SO, D], BF16, tag="V")     # [t, d_out]

            # ---------------- phase 1: load + transpose + projections -------
            with (
                tc.tile_pool(name="wpool", bufs=2) as wpool,
                tc.tile_pool(name="stage", bufs=3) as stage,
                tc.tile_pool(name="ph1", bufs=1) as ph1,
            ):
                if with_bias:
                    ones = ph1.tile([1, NS], BF16, tag="ones")
                    nc.vector.memset(ones[:], 1.0)
                    b_sb = {}
                    for nm in ("q", "k", "v"):
                        bs = stage.tile([1, D], F32, tag="bstage")
                        nc.sync.dma_start(bs[:], b_ext[nm].ap())
                        bt = ph1.tile([1, D], BF16, tag=f"b{nm}")
                        nc.vector.tensor_copy(out=bt[:], in_=bs[:])
                        b_sb[nm] = bt

                def load_w(nm):
                    wt = wpool.tile([P, DO, D], BF16, tag="w")
                    for ko in range(DO):
                        st = stage.tile([P, D], F32, tag="wstage")
                        nc.sync.dma_start(
                            st[:], w_ext[nm].ap()[ko * P:(ko + 1) * P, :])
                        nc.vector.tensor_copy(out=wt[:, ko, :], in_=st[:])
                    return wt

                wq_sb = load_w("q")

                # x -> bf16 -> xT [d, s]
                xT = ph1.tile([P, DO, S], BF16, tag="xT")
                for si in range(SO):
                    st = stage.tile([P, D], F32, tag="xstage")
                    nc.sync.dma_start(
                        st[:], x_ext.ap()[si * P:(si + 1) * P, :])
                    xb = stage.tile([P, D], BF16, tag="xbf")
                    nc.vector.tensor_copy(out=xb[:], in_=st[:])
                    for ko in range(DO):
                        pt = psum_tr.tile([P, P], BF16, tag="tr")
                        nc.tensor.transpose(
                            pt[:], xb[:, ko * P:(ko + 1) * P], ident[:])
                        nc.vector.tensor_copy(
                            out=xT[:, ko, si * P:(si + 1) * P], in_=pt[:])

                wk_sb = load_w("k")

                # QT / KT projections: psum[d_out 128, s 512]
                def proj_t(dst, w, nm):
                    for mo in range(DO):
                        for no in range(S // NS):
                            ps = psum_mm.tile([P, NS], F32, tag="mm")
                            for k in range(DO):
                                nc.tensor.matmul(
                                    ps[:],
                                    w[:, k, mo * P:(mo + 1) * P],
                                    xT[:, k, no * NS:(no + 1) * NS],
                                    start=(k == 0), stop=(k == DO - 1),
                                )
                            if with_bias:
                                # psum[do,s] += b[do] x ones[s]  (K=1 matmul)
                                bl = b_sb[nm].rearrange("o d -> d o")
                                nc.tensor.matmul(
                                    ps[:], bl[mo * P:(mo + 1) * P, :],
                                    ones[:], start=False, stop=True,
                                    skip_group_check=True,
                                )
                            nc.scalar.copy(
                                out=dst[:, mo, no * NS:(no + 1) * NS],
                                in_=ps[:])

                proj_t(QT, wq_sb, "q")
                wv_sb = load_w("v")
                proj_t(KT, wk_sb, "k")

                # V projection: psum[t 128, d_out 512]
                for to in range(SO):
                    for no in range(D // NS):
                        ps = psum_mm.tile([P, NS], F32, tag="mm")
                        for k in range(DO):
                            nc.tensor.matmul(
                                ps[:],
                                xT[:, k, to * P:(to + 1) * P],
                                wv_sb[:, k, no * NS:(no + 1) * NS],
                                start=(k == 0), stop=(k == DO - 1),
                            )
                        if with_bias:
                            ob = ones.rearrange("o d -> d o")
                            nc.tensor.matmul(
                                ps[:], ob[:P, :],
                                b_sb["v"][:, no * NS:(no + 1) * NS],
                                start=False, stop=True,
                                skip_group_check=True,
                            )
                        nc.scalar.copy(
                            out=V[:, to, no * NS:(no + 1) * NS], in_=ps[:])

            # ---------------- phase 2: attention ----------------------------
            with tc.tile_pool(name="attnpool", bufs=2) as work:
                for qi in range(SO):
                    attn = work.tile([P, S], BF16, tag="attn")
                    ssum = work.tile([P, S // NS], F32, tag="ssum")
                    for tj in range(S // NS):
                        ps = psum_mm.tile([P, NS], F32, tag="mm")
                        for k in range(DO):
                            nc.tensor.matmul(
                                ps[:],
                                QT[:, k, qi * P:(qi + 1) * P],
                                KT[:, k, tj * NS:(tj + 1) * NS],
                                start=(k == 0), stop=(k == DO - 1),
                            )
                        # attn = exp(scores/32), row-sums accumulated free
                        nc.scalar.activation(
                            out=attn[:, tj * NS:(tj + 1) * NS], in_=ps[:],
                            func=mybir.ActivationFunctionType.Exp,
                            scale=SCALE,
                            accum_out=ssum[:, tj:tj + 1],
                        )
                    tsum = work.tile([P, 1], F32, tag="tsum")
                    nc.vector.reduce_sum(
                        tsum[:], ssum[:], axis=mybir.AxisListType.X)
                    rsum = work.tile([P, 1], F32, tag="rsum")
                    nc.vector.reciprocal(rsum[:], tsum[:])

                    attnT = work.tile([P, SO, P], BF16, tag="attnT")
                    for tj in range(SO):
                        pt = psum_tr.tile([P, P], BF16, tag="tr")
                        nc.tensor.transpose(
                            pt[:], attn[:, tj * P:(tj + 1) * P], ident[:])
                        nc.vector.tensor_copy(out=attnT[:, tj, :], in_=pt[:])

                    for do in range(D // NS):
                        ps = psum_tr.tile([P, NS], F32, tag="av")
                        for tj in range(SO):
                            nc.tensor.matmul(
                                ps[:],
                                attnT[:, tj, :],
                                V[:, tj, do * NS:(do + 1) * NS],
                                start=(tj == 0), stop=(tj == SO - 1),
                            )
                        ot = work.tile([P, NS], F32, tag="ot")
                        nc.scalar.mul(out=ot[:], in_=ps[:], mul=rsum[:])
                        nc.sync.dma_start(
                            out_ext.ap()[qi * P:(qi + 1) * P,
                                         do * NS:(do + 1) * NS],
                            ot[:])

    nc.compile()
    return nc


_cache = {}


def _get(with_bias: bool):
    if with_bias not in _cache:
        _cache[with_bias] = build(with_bias)
    return _cache[with_bias]


def _run(x, Wq, bq, Wk, bk, Wv, bv, trace=False, tmpdir=None):
    x = np.ascontiguousarray(np.asarray(x, dtype=np.float32))
    Wq = np.ascontiguousarray(np.asarray(Wq, dtype=np.float32))
    Wk = np.ascontiguousarray(np.asarray(Wk, dtype=np.float32))
    Wv = np.ascontiguousarray(np.asarray(Wv, dtype=np.float32))
    bq = np.ascontiguousarray(np.asarray(bq, dtype=np.float32)).reshape(1, D)
    bk = np.ascontiguousarray(np.asarray(bk, dtype=np.float32)).reshape(1, D)
    bv = np.ascontiguousarray(np.asarray(bv, dtype=np.float32)).reshape(1, D)
    with_bias = bool(np.any(bq) or np.any(bk) or np.any(bv))
    nc = _get(with_bias)
    in_maps = [
        {"x": x[i], "Wq": Wq, "Wk": Wk, "Wv": Wv, "bq": bq, "bk": bk, "bv": bv}
        for i in range(B)
    ]
    res = run_bass_kernel_spmd(
        nc, in_maps, core_ids=list(range(N_CORES)), trace=trace, tmpdir=tmpdir)
    out = np.stack([res.results[i]["out"] for i in range(B)], axis=0)
    return out.astype(np.float32, copy=False), res


def kernel(x, Wq, bq, Wk, bk, Wv, bv):
    out, _ = _run(x, Wq, bq, Wk, bk, Wv, bv)
    return out
